# revision 33
# baseline (speedup 1.0000x reference)
"""Trainium2 Bass kernel for zonotope AbstractRelu (nn_AbstractRelu_76751065579631).

Problem: x [E=512, D1=4096, D2=16] f32. Per neuron column n (N = D1*D2 = 65536):
    sum_abs[n] = sum_{e>=1} |x[e, n]|
    lb = x[0] - sum_abs ; ub = x[0] + sum_abs
    scale = (ub > 0) * (1 - min(lb, 0))
    out[0]   = scale * (x[0] - min(lb, 0)/2)
    out[1:]  = scale * x[1:]
(algebraically identical to the reference's crossing/dead formulation)

Sharding: neuron columns split contiguously across 8 cores (8192 each), no
communication.

Precision/traffic (tolerance 2e-2, measured ~1.4e-3): error rows travel bf16
both ways (host casts); the center row stays f32 (it decides the crossing/
dead classification and carries ~98% of output energy). 16.9 MB HBM traffic
per core -> ~46 us DMA floor at ~23 GB/s x 16 DMA engines.

Measured engine facts baked into this layout (from neuron-profile traces):
 - a dma_start costs the issuing engine 0.65-2.4us -> ONE consolidated load
   per chunk (partition-major [128, NB, W] view) on the otherwise-idle SP
   ring; ONE consolidated store per chunk on the Pool SWDGE ring;
 - matmul to PSUM base partition 0 runs ~430ns per 512-col piece; bases
   32/64 cost ~630ns -> reduce accumulates into plain [1, W] psum tiles;
 - DVE tensor ops are ~1 elem/cycle/lane regardless of dtype; in-place
   multiplies (out==in0) run 685ns vs ~2.1us for 3-stream -> multiply in
   place over xt; fp8 output from DVE/Pool is 3x slower -> bf16 out;
 - ACT activation ~1.07ns/col: abs split ACT [0:2304] / DVE [2304:4096]
   balances the two; psum evacuation ([1,W] row copies) + psum_b -> bf16
   bc copies also live on ACT;
 - scale path runs once per super-chunk of SC=4 chunks on a [128, 32]
   repartition (128B DMA runs, tiny-packet overhead amortized).
The scale/broadcast/multiply/store chain is emitted under tc.high_priority
so the Tile scheduler starts the store stream while loads are still going.
"""

import os

import numpy as np

E = 512
D1 = 4096
D2 = 16
N = D1 * D2          # 65536 neurons
NCORES = 8
COLS = N // NCORES   # 8192 neuron columns per core
W = 1024             # chunk width
SC = 4               # chunks per super-chunk (scale-path granularity)

LAST_EXEC_TIME_NS = None

_CACHE = {}


def _emit(tc, oe_ap, oc_ap, xe_ap, xc_ap, W, SC):
    import concourse.mybir as mybir

    nc = tc.nc
    f32 = mybir.dt.float32
    bf16 = mybir.dt.bfloat16
    Alu = mybir.AluOpType
    Act = mybir.ActivationFunctionType

    e_total, cols = xe_ap.shape
    NB = e_total // 128          # e-blocks of 128 partitions
    NCH = cols // W              # chunks
    ABS_ACT = (NB * W * 11) // 16  # abs split point (ACT share)

    # graded super-chunk sizes: the first scale phases cover few chunks so
    # the multiply/store stream starts while loads are still in flight;
    # later ones grow to amortize the scale-path overhead
    SCS = []
    left, step = NCH, 1
    while left > 0:
        s = min(step, left)
        SCS.append(s)
        left -= s
        if step < SC:
            step *= 2

    # partition-major DRAM views: one dma_start per chunk
    x_pbn = xe_ap.rearrange("(b p) n -> p b n", p=128)
    o_pbn = oe_ap.rearrange("(b p) n -> p b n", p=128)

    with (
        tc.tile_pool(name="const", bufs=1) as const_pool,
        tc.tile_pool(name="x", bufs=8) as x_pool,
        tc.tile_pool(name="abs", bufs=4) as abs_pool,
        tc.tile_pool(name="row", bufs=2) as row_pool,
        tc.tile_pool(name="small", bufs=2) as small_pool,
        tc.tile_pool(name="bc", bufs=3) as bc_pool,
        tc.tile_pool(name="psum_s", bufs=2, space="PSUM") as psum_s_pool,
    ):
        ones_row = const_pool.tile([1, 128], bf16, tag="ones_row")
        nc.vector.memset(ones_row[:], 1.0)
        ones_col = const_pool.tile([128, 1], bf16, tag="ones_col")
        nc.vector.memset(ones_col[:], 1.0)

        def pieces(Wk):
            return [(ps, min(512, Wk - ps)) for ps in range(0, Wk, 512)]

        def front(cs, s_sc, koff):
            """One chunk load (SP ring), |x| split ACT/DVE, partition-sum
            matmuls (PE), psum -> s_sc row copy (ACT)."""
            st = {"cs": cs}
            xt = x_pool.tile([128, NB * W], bf16, tag="x")
            nc.sync.dma_start(out=xt[:], in_=x_pbn[:, :, cs:cs + W])

            at = abs_pool.tile([128, NB * W], bf16, tag="a")
            nc.scalar.activation(at[:, 0:ABS_ACT], xt[:, 0:ABS_ACT], Act.Abs)
            # DVE abs: max(-x, x) in one scalar_tensor_tensor
            nc.vector.scalar_tensor_tensor(
                at[:, ABS_ACT:], in0=xt[:, ABS_ACT:], scalar=-1.0,
                in1=xt[:, ABS_ACT:], op0=Alu.mult, op1=Alu.max,
            )
            psum_s = psum_s_pool.tile([1, W], f32, tag="s")
            for ps, pw in pieces(W):
                for b in range(NB):
                    nc.tensor.matmul(
                        psum_s[0:1, ps:ps + pw],
                        lhsT=ones_col[:],
                        rhs=at[:, b * W + ps:b * W + ps + pw],
                        start=(b == 0),
                        stop=(b == NB - 1),
                    )
            # free psum_s early: copy into the super-chunk row (ACT)
            nc.scalar.copy(s_sc[0:1, koff * W:(koff + 1) * W], psum_s[:])
            st.update(xt=xt)
            return st

        def scale_phase(cs, scw, s_sc, tg):
            """Per-super-chunk scale math on the [128, scw/128] repartition."""
            WP = scw // 128
            # repartition row -> [128, WP] and center row load (Pool SWDGE;
            # keeps the ACT/SP instruction streams free)
            s_t = small_pool.tile([128, WP], f32, tag=f"st{tg}", name="s_t")
            nc.gpsimd.dma_start(out=s_t[:], in_=s_sc[:])
            c_t = small_pool.tile([128, WP], f32, tag=f"ct{tg}", name="c_t")
            nc.gpsimd.dma_start(out=c_t[:], in_=xc_ap[0:1, cs:cs + scw])

            g = nc.vector

            def sm(tag):
                return small_pool.tile([128, WP], f32, tag=f"{tag}{tg}",
                                       name=tag)

            lb = sm("lb")
            g.tensor_sub(lb[:], c_t[:], s_t[:])
            ub = sm("ub")
            g.tensor_add(ub[:], c_t[:], s_t[:])
            min0 = sm("min0")
            g.tensor_scalar_min(min0[:], lb[:], 0.0)
            alpha = sm("alpha")
            g.tensor_scalar(alpha[:], min0[:], -1.0, 1.0, Alu.mult, Alu.add)
            gt = sm("gt")
            g.tensor_scalar(gt[:], ub[:], 0.0, None, Alu.is_gt)
            scale = sm("scale")
            g.tensor_mul(scale[:], alpha[:], gt[:])
            scale_bf = small_pool.tile([128, WP], bf16, tag=f"scalebf{tg}",
                                       name="scale_bf")
            g.tensor_mul(scale_bf[:], alpha[:], gt[:])

            # scale back to row layout for the K=1 broadcast matmuls
            scale_row = row_pool.tile([1, scw], bf16, tag=f"scrow{tg}",
                                      name="scale_row")
            nc.gpsimd.dma_start(out=scale_row[:], in_=scale_bf[:])

            t1 = sm("t1")
            g.scalar_tensor_tensor(t1[:], in0=min0[:], scalar=-0.5,
                                   in1=c_t[:], op0=Alu.mult, op1=Alu.add)
            cnew = sm("cnew")
            g.tensor_mul(cnew[:], t1[:], scale[:])
            # center output: [128, WP] -> DRAM row (reverse repartition)
            nc.gpsimd.dma_start(out=oc_ap[0:1, cs:cs + scw], in_=cnew[:])
            return scale_row

        def backmul(st, scale_row, koff):
            """Broadcast scale across partitions (one GpSimd
            partition_broadcast, no PSUM round-trip), multiply the 4
            x-blocks in place (DVE)."""
            xt = st["xt"]
            bc = bc_pool.tile([128, W], bf16, tag="bc")
            nc.gpsimd.partition_broadcast(
                bc[:], scale_row[0:1, koff * W:(koff + 1) * W])
            for b in range(NB):
                nc.vector.tensor_mul(xt[:, b * W:(b + 1) * W],
                                     xt[:, b * W:(b + 1) * W], bc[:])

        def back2(st):
            """One consolidated chunk store (Pool SWDGE queue)."""
            cs, xt = st["cs"], st["xt"]
            nc.gpsimd.dma_start(out=o_pbn[:, :, cs:cs + W], in_=xt[:])

        ks = 0
        for j, scj in enumerate(SCS):
            scw = scj * W
            s_sc = row_pool.tile([1, scw], f32, tag=f"s_sc{scj}",
                                 name="s_sc")
            sts = [front((ks + i) * W, s_sc, i) for i in range(scj)]
            with tc.high_priority():
                scale_row = scale_phase(ks * W, scw, s_sc, scj)
                for i in range(scj):
                    backmul(sts[i], scale_row, i)
                    back2(sts[i])
            ks += scj


def build(cols=COLS, e_total=E, w=W, sc=SC):
    """Build + compile the per-core Bass program (cached)."""
    key = (cols, e_total, w, sc)
    if key in _CACHE:
        return _CACHE[key]

    from concourse import bacc
    import concourse.mybir as mybir
    from concourse.tile import TileContext

    nc = bacc.Bacc("TRN2", target_bir_lowering=False, debug=False,
                   num_devices=NCORES)
    xe_ap = nc.dram_tensor("xe", [e_total, cols], mybir.dt.bfloat16,
                           kind="ExternalInput").ap()
    xc_ap = nc.dram_tensor("xc", [1, cols], mybir.dt.float32,
                           kind="ExternalInput").ap()
    oe_ap = nc.dram_tensor("oe", [e_total, cols], mybir.dt.bfloat16,
                           kind="ExternalOutput").ap()
    oc_ap = nc.dram_tensor("oc", [1, cols], mybir.dt.float32,
                           kind="ExternalOutput").ap()
    with TileContext(nc) as tc:
        _emit(tc, oe_ap, oc_ap, xe_ap, xc_ap, w, sc)
    nc.compile()
    _CACHE[key] = nc
    return nc


def _ensure_ntff_hook():
    """Install the axon NTFF profile hook when the image's antenv lacks it."""
    import sys
    import types

    try:
        from antenv.axon_hooks import get_axon_ntff_profile_hook  # noqa: F401
        return
    except ImportError:
        pass

    mod = types.ModuleType("antenv.axon_hooks")
    mod._hook = None

    def set_axon_ntff_profile_hook(h):
        mod._hook = h

    def get_axon_ntff_profile_hook():
        return mod._hook

    mod.set_axon_ntff_profile_hook = set_axon_ntff_profile_hook
    mod.get_axon_ntff_profile_hook = get_axon_ntff_profile_hook
    sys.modules["antenv.axon_hooks"] = mod
    import antenv

    antenv.axon_hooks = mod
    try:
        from trn_agent_boot.trn_boot import _ntff_profile_via_ctypes

        set_axon_ntff_profile_hook(
            _ntff_profile_via_ctypes("/opt/axon/libaxon_pjrt.so")
        )
    except Exception:
        pass


def kernel(x):
    global LAST_EXEC_TIME_NS
    import ml_dtypes
    from concourse import bass_utils

    nc = build()
    xf = np.asarray(x, dtype=np.float32).reshape(E, N)
    xe = xf.astype(ml_dtypes.bfloat16)
    xe[0] = 0  # center row excluded from the |.| reduce
    in_maps = []
    for c in range(NCORES):
        sl = slice(c * COLS, (c + 1) * COLS)
        in_maps.append({
            "xe": np.ascontiguousarray(xe[:, sl]),
            "xc": np.ascontiguousarray(xf[0:1, sl]),
        })
    trace = bool(int(os.environ.get("KERNEL_TRACE", "0")))
    if trace:
        _ensure_ntff_hook()
        # Sandboxed container: keep profile artifacts local.
        bass_utils.upload_artifacts = lambda tmpdir: tmpdir
    res = bass_utils.run_bass_kernel_spmd(
        nc, in_maps, core_ids=list(range(NCORES)), trace=trace
    )
    LAST_EXEC_TIME_NS = res.exec_time_ns
    out = np.empty((E, N), dtype=np.float32)
    for c in range(NCORES):
        sl = slice(c * COLS, (c + 1) * COLS)
        out[1:, sl] = res.results[c]["oe"][1:].astype(np.float32)
        out[0, sl] = res.results[c]["oc"][0]
    return out.reshape(E, D1, D2)


# revision 34
# speedup vs baseline: 1.1259x; 1.1259x over previous
"""Trainium2 Bass kernel for zonotope AbstractRelu (nn_AbstractRelu_76751065579631).

Problem: x [E=512, D1=4096, D2=16] f32. Per neuron column n (N = D1*D2 = 65536):
    sum_abs[n] = sum_{e>=1} |x[e, n]|
    lb = x[0] - sum_abs ; ub = x[0] + sum_abs
    scale = (ub > 0) * (1 - min(lb, 0))
    out[0]   = scale * (x[0] - min(lb, 0)/2)
    out[1:]  = scale * x[1:]
(algebraically identical to the reference's crossing/dead formulation)

Sharding: neuron columns split contiguously across 8 cores (8192 each), no
communication.

Precision/traffic (tolerance 2e-2, measured ~1.4e-3): error rows travel bf16
both ways (host casts); the center row stays f32 (it decides the crossing/
dead classification and carries ~98% of output energy). 16.9 MB HBM traffic
per core -> ~46 us DMA floor at ~23 GB/s x 16 DMA engines.

Measured engine facts baked into this layout (from neuron-profile traces):
 - a dma_start costs the issuing engine 0.65-2.4us -> ONE consolidated load
   per chunk (partition-major [128, NB, W] view) on the otherwise-idle SP
   ring; ONE consolidated store per chunk on the Pool SWDGE ring;
 - matmul to PSUM base partition 0 runs ~430ns per 512-col piece; bases
   32/64 cost ~630ns -> reduce accumulates into plain [1, W] psum tiles;
 - DVE tensor ops are ~1 elem/cycle/lane regardless of dtype; in-place
   multiplies (out==in0) run 685ns vs ~2.1us for 3-stream -> multiply in
   place over xt; fp8 output from DVE/Pool is 3x slower -> bf16 out;
 - ACT activation ~1.07ns/col: abs split ACT [0:2304] / DVE [2304:4096]
   balances the two; psum evacuation ([1,W] row copies) + psum_b -> bf16
   bc copies also live on ACT;
 - scale path runs once per super-chunk of SC=4 chunks on a [128, 32]
   repartition (128B DMA runs, tiny-packet overhead amortized).
The scale/broadcast/multiply/store chain is emitted under tc.high_priority
so the Tile scheduler starts the store stream while loads are still going.
"""

import os

import numpy as np

E = 512
D1 = 4096
D2 = 16
N = D1 * D2          # 65536 neurons
NCORES = 8
COLS = N // NCORES   # 8192 neuron columns per core
W = 1024             # chunk width
SC = 4               # chunks per super-chunk (scale-path granularity)

LAST_EXEC_TIME_NS = None

_CACHE = {}


def _emit(tc, oe_ap, oc_ap, xe_ap, xc_ap, W, SC):
    import concourse.mybir as mybir

    nc = tc.nc
    f32 = mybir.dt.float32
    bf16 = mybir.dt.bfloat16
    Alu = mybir.AluOpType
    Act = mybir.ActivationFunctionType

    e_total, cols = xe_ap.shape
    NB = e_total // 128          # e-blocks of 128 partitions
    NCH = cols // W              # chunks
    ABS_ACT = (NB * W * 7) // 8  # abs split point (ACT share)

    # graded super-chunk sizes: the first scale phases cover few chunks so
    # the multiply/store stream starts while loads are still in flight;
    # later ones grow to amortize the scale-path overhead
    SCS = []
    left, step = NCH, 1
    while left > 0:
        s = min(step, left)
        SCS.append(s)
        left -= s
        if step < SC:
            step *= 2

    # partition-major DRAM views: one dma_start per chunk
    x_pbn = xe_ap.rearrange("(b p) n -> p b n", p=128)
    o_pbn = oe_ap.rearrange("(b p) n -> p b n", p=128)

    with (
        tc.tile_pool(name="const", bufs=1) as const_pool,
        tc.tile_pool(name="x", bufs=8) as x_pool,
        tc.tile_pool(name="abs", bufs=4) as abs_pool,
        tc.tile_pool(name="row", bufs=2) as row_pool,
        tc.tile_pool(name="small", bufs=2) as small_pool,
        tc.tile_pool(name="bc", bufs=3) as bc_pool,
        tc.tile_pool(name="psum_s", bufs=2, space="PSUM") as psum_s_pool,
        tc.tile_pool(name="psum_b", bufs=2, space="PSUM") as psum_b_pool,
    ):
        ones_row = const_pool.tile([1, 128], bf16, tag="ones_row")
        nc.vector.memset(ones_row[:], 1.0)
        ones_col = const_pool.tile([128, 1], bf16, tag="ones_col")
        nc.vector.memset(ones_col[:], 1.0)

        def pieces(Wk):
            return [(ps, min(512, Wk - ps)) for ps in range(0, Wk, 512)]

        def front(cs, s_sc, koff):
            """One chunk load (SP ring), |x| split ACT/DVE, partition-sum
            matmuls (PE), psum -> s_sc row copy (ACT)."""
            st = {"cs": cs}
            xt = x_pool.tile([128, NB * W], bf16, tag="x")
            nc.sync.dma_start(out=xt[:], in_=x_pbn[:, :, cs:cs + W])

            at = abs_pool.tile([128, NB * W], bf16, tag="a")
            nc.scalar.activation(at[:, 0:ABS_ACT], xt[:, 0:ABS_ACT], Act.Abs)
            # DVE abs: max(-x, x) in one scalar_tensor_tensor
            nc.vector.scalar_tensor_tensor(
                at[:, ABS_ACT:], in0=xt[:, ABS_ACT:], scalar=-1.0,
                in1=xt[:, ABS_ACT:], op0=Alu.mult, op1=Alu.max,
            )
            psum_s = psum_s_pool.tile([1, W], f32, tag="s")
            for ps, pw in pieces(W):
                for b in range(NB):
                    nc.tensor.matmul(
                        psum_s[0:1, ps:ps + pw],
                        lhsT=ones_col[:],
                        rhs=at[:, b * W + ps:b * W + ps + pw],
                        start=(b == 0),
                        stop=(b == NB - 1),
                    )
            # free psum_s early: copy into the super-chunk row (ACT)
            nc.scalar.copy(s_sc[0:1, koff * W:(koff + 1) * W], psum_s[:])
            st.update(xt=xt)
            return st

        def scale_phase(cs, scw, s_sc, tg):
            """Per-super-chunk scale math on the [128, scw/128] repartition."""
            WP = scw // 128
            # repartition row -> [128, WP] and center row load (Pool SWDGE;
            # keeps the ACT/SP instruction streams free)
            s_t = small_pool.tile([128, WP], f32, tag=f"st{tg}", name="s_t")
            nc.gpsimd.dma_start(out=s_t[:], in_=s_sc[:])
            c_t = small_pool.tile([128, WP], f32, tag=f"ct{tg}", name="c_t")
            nc.gpsimd.dma_start(out=c_t[:], in_=xc_ap[0:1, cs:cs + scw])

            g = nc.vector

            def sm(tag):
                return small_pool.tile([128, WP], f32, tag=f"{tag}{tg}",
                                       name=tag)

            lb = sm("lb")
            g.tensor_sub(lb[:], c_t[:], s_t[:])
            ub = sm("ub")
            g.tensor_add(ub[:], c_t[:], s_t[:])
            min0 = sm("min0")
            g.tensor_scalar_min(min0[:], lb[:], 0.0)
            alpha = sm("alpha")
            g.tensor_scalar(alpha[:], min0[:], -1.0, 1.0, Alu.mult, Alu.add)
            gt = sm("gt")
            g.tensor_scalar(gt[:], ub[:], 0.0, None, Alu.is_gt)
            scale = sm("scale")
            g.tensor_mul(scale[:], alpha[:], gt[:])
            scale_bf = small_pool.tile([128, WP], bf16, tag=f"scalebf{tg}",
                                       name="scale_bf")
            g.tensor_mul(scale_bf[:], alpha[:], gt[:])

            # scale back to row layout for the K=1 broadcast matmuls
            scale_row = row_pool.tile([1, scw], bf16, tag=f"scrow{tg}",
                                      name="scale_row")
            nc.gpsimd.dma_start(out=scale_row[:], in_=scale_bf[:])

            t1 = sm("t1")
            g.scalar_tensor_tensor(t1[:], in0=min0[:], scalar=-0.5,
                                   in1=c_t[:], op0=Alu.mult, op1=Alu.add)
            cnew = sm("cnew")
            g.tensor_mul(cnew[:], t1[:], scale[:])
            # center output: [128, WP] -> DRAM row (reverse repartition)
            nc.gpsimd.dma_start(out=oc_ap[0:1, cs:cs + scw], in_=cnew[:])
            return scale_row

        def backmul(st, scale_row, koff):
            """Broadcast scale across partitions (K=1 ones matmul into
            PSUM), multiply the 4 x-blocks in place (DVE reads psum_b
            directly -- one less engine hop than staging a bf16 copy)."""
            xt = st["xt"]
            psum_b = psum_b_pool.tile([128, W], f32, tag="b")
            for ps, pw in pieces(W):
                nc.tensor.matmul(
                    psum_b[:, ps:ps + pw],
                    lhsT=ones_row[:],
                    rhs=scale_row[0:1, koff * W + ps:koff * W + ps + pw],
                    start=True,
                    stop=True,
                )
            for b in range(NB):
                nc.vector.tensor_mul(xt[:, b * W:(b + 1) * W],
                                     xt[:, b * W:(b + 1) * W], psum_b[:])

        def back2(st):
            """One consolidated chunk store (Pool SWDGE queue)."""
            cs, xt = st["cs"], st["xt"]
            nc.gpsimd.dma_start(out=o_pbn[:, :, cs:cs + W], in_=xt[:])

        ks = 0
        for j, scj in enumerate(SCS):
            scw = scj * W
            s_sc = row_pool.tile([1, scw], f32, tag=f"s_sc{scj}",
                                 name="s_sc")
            sts = [front((ks + i) * W, s_sc, i) for i in range(scj)]
            with tc.high_priority():
                scale_row = scale_phase(ks * W, scw, s_sc, scj)
                for i in range(scj):
                    backmul(sts[i], scale_row, i)
                    back2(sts[i])
            ks += scj


def build(cols=COLS, e_total=E, w=W, sc=SC):
    """Build + compile the per-core Bass program (cached)."""
    key = (cols, e_total, w, sc)
    if key in _CACHE:
        return _CACHE[key]

    from concourse import bacc
    import concourse.mybir as mybir
    from concourse.tile import TileContext

    nc = bacc.Bacc("TRN2", target_bir_lowering=False, debug=False,
                   num_devices=NCORES)
    xe_ap = nc.dram_tensor("xe", [e_total, cols], mybir.dt.bfloat16,
                           kind="ExternalInput").ap()
    xc_ap = nc.dram_tensor("xc", [1, cols], mybir.dt.float32,
                           kind="ExternalInput").ap()
    oe_ap = nc.dram_tensor("oe", [e_total, cols], mybir.dt.bfloat16,
                           kind="ExternalOutput").ap()
    oc_ap = nc.dram_tensor("oc", [1, cols], mybir.dt.float32,
                           kind="ExternalOutput").ap()
    with TileContext(nc) as tc:
        _emit(tc, oe_ap, oc_ap, xe_ap, xc_ap, w, sc)
    nc.compile()
    _CACHE[key] = nc
    return nc


def _ensure_ntff_hook():
    """Install the axon NTFF profile hook when the image's antenv lacks it."""
    import sys
    import types

    try:
        from antenv.axon_hooks import get_axon_ntff_profile_hook  # noqa: F401
        return
    except ImportError:
        pass

    mod = types.ModuleType("antenv.axon_hooks")
    mod._hook = None

    def set_axon_ntff_profile_hook(h):
        mod._hook = h

    def get_axon_ntff_profile_hook():
        return mod._hook

    mod.set_axon_ntff_profile_hook = set_axon_ntff_profile_hook
    mod.get_axon_ntff_profile_hook = get_axon_ntff_profile_hook
    sys.modules["antenv.axon_hooks"] = mod
    import antenv

    antenv.axon_hooks = mod
    try:
        from trn_agent_boot.trn_boot import _ntff_profile_via_ctypes

        set_axon_ntff_profile_hook(
            _ntff_profile_via_ctypes("/opt/axon/libaxon_pjrt.so")
        )
    except Exception:
        pass


def kernel(x):
    global LAST_EXEC_TIME_NS
    import ml_dtypes
    from concourse import bass_utils

    nc = build()
    xf = np.asarray(x, dtype=np.float32).reshape(E, N)
    xe = xf.astype(ml_dtypes.bfloat16)
    xe[0] = 0  # center row excluded from the |.| reduce
    in_maps = []
    for c in range(NCORES):
        sl = slice(c * COLS, (c + 1) * COLS)
        in_maps.append({
            "xe": np.ascontiguousarray(xe[:, sl]),
            "xc": np.ascontiguousarray(xf[0:1, sl]),
        })
    trace = bool(int(os.environ.get("KERNEL_TRACE", "0")))
    if trace:
        _ensure_ntff_hook()
        # Sandboxed container: keep profile artifacts local.
        bass_utils.upload_artifacts = lambda tmpdir: tmpdir
    res = bass_utils.run_bass_kernel_spmd(
        nc, in_maps, core_ids=list(range(NCORES)), trace=trace
    )
    LAST_EXEC_TIME_NS = res.exec_time_ns
    out = np.empty((E, N), dtype=np.float32)
    for c in range(NCORES):
        sl = slice(c * COLS, (c + 1) * COLS)
        out[1:, sl] = res.results[c]["oe"][1:].astype(np.float32)
        out[0, sl] = res.results[c]["oc"][0]
    return out.reshape(E, D1, D2)
